# revision 1
# baseline (speedup 1.0000x reference)
"""Trainium2 Bass kernel for nn_SoftBiasTransformer.

3-layer post-norm transformer encoder, B=1024 S=64 D=768 H=6 HD=128 FF=3072,
with a learned [S,S] additive attention bias shared across batch/heads.

Strategy:
- Data-parallel over batch across 8 NeuronCores (128 batches/core).
- fp16 matmul operands (full TensorE rate, ~3e-4 rel precision), fp32 PSUM
  accumulation, fp32 layernorm statistics.
- Feature-major activations [D(part), tokens(free)]: dense chain computes
  outT = W.T @ xT with weights stationary, no activation transposes. V is
  produced token-major (x stationary) so the attention context matmul can
  consume PE-transposed softmax probabilities directly.
- LayerNorm mean/var via (1/D)-vector matmuls on TensorE (reduction over
  the feature/partition axis), broadcast back with K=1 matmuls.
- Softmax bias folded in multiplicatively: exp(s+b) = exp(s)*exp(b), with
  exp(bias) precomputed on the host from sp_table[sp_matrix]. The 1/sqrt(HD)
  score scale is folded into Wq/bq on the host.
"""

import math

import numpy as np

B, S, D = 1024, 64, 768
H, HD, FF, L = 6, 128, 3072, 3
NCORES = 8
BS = B // NCORES            # batches per core = 128
TOK = BS * S                # tokens per core = 8192
KD = D // 128               # 6
KF = FF // 128              # 24
EPS = 1e-5
TC = 1024                   # tokens per chunk
BC = TC // S                # batches per chunk = 16
NQ = TC // 512              # 512-col units per chunk = 2
T8 = TC // 128              # 128-token blocks per chunk = 8

_CACHED_NC = {}


def _build_nc(n_chunks, stage="full"):
    import concourse.tile as tile
    import concourse.mybir as mybir
    from concourse import bacc
    from contextlib import ExitStack

    f16 = mybir.dt.float16
    f32 = mybir.dt.float32
    Alu = mybir.AluOpType
    Act = mybir.ActivationFunctionType

    nc = bacc.Bacc("TRN2", target_bir_lowering=False, debug=False,
                   enable_asserts=False, num_devices=1)

    # ---- DRAM I/O ----
    xw = nc.dram_tensor("xw", [KD, 128, TOK], f16, kind="ExternalInput")
    Wq_s = nc.dram_tensor("Wq_s", [L, KD, 128, D], f16, kind="ExternalInput")
    Wk_s = nc.dram_tensor("Wk_s", [L, KD, 128, D], f16, kind="ExternalInput")
    Wv_s = nc.dram_tensor("Wv_s", [L, KD, 128, D], f16, kind="ExternalInput")
    Wo_s = nc.dram_tensor("Wo_s", [L, KD, 128, D], f16, kind="ExternalInput")
    W1_s = nc.dram_tensor("W1_s", [L, KD, 128, FF], f16, kind="ExternalInput")
    W2_s = nc.dram_tensor("W2_s", [L, KF, 128, D], f16, kind="ExternalInput")
    bq_t = nc.dram_tensor("bq_t", [L, 128, KD], f32, kind="ExternalInput")
    bk_t = nc.dram_tensor("bk_t", [L, 128, KD], f32, kind="ExternalInput")
    bv_t = nc.dram_tensor("bv_t", [L, 128, KD], f32, kind="ExternalInput")
    bo_t = nc.dram_tensor("bo_t", [L, 128, KD], f32, kind="ExternalInput")
    b1_t = nc.dram_tensor("b1_t", [L, 128, KF], f32, kind="ExternalInput")
    b2_t = nc.dram_tensor("b2_t", [L, 128, KD], f32, kind="ExternalInput")
    l1s_t = nc.dram_tensor("l1s_t", [L, 128, KD], f32, kind="ExternalInput")
    l1b_t = nc.dram_tensor("l1b_t", [L, 128, KD], f32, kind="ExternalInput")
    l2s_t = nc.dram_tensor("l2s_t", [L, 128, KD], f32, kind="ExternalInput")
    l2b_t = nc.dram_tensor("l2b_t", [L, 128, KD], f32, kind="ExternalInput")
    eb2 = nc.dram_tensor("eb2", [128, S], f16, kind="ExternalInput")
    id16 = nc.dram_tensor("id16", [128, 128], f16, kind="ExternalInput")
    y = nc.dram_tensor("y", [TOK, D], f32, kind="ExternalOutput")

    inv_d = 1.0 / D

    with tile.TileContext(nc) as tc, ExitStack() as ctx:
        consts = ctx.enter_context(tc.tile_pool(name="consts", bufs=1))
        p_x = ctx.enter_context(tc.tile_pool(name="p_x", bufs=1))
        p_act = ctx.enter_context(tc.tile_pool(name="p_act", bufs=1))
        p_sm = ctx.enter_context(tc.tile_pool(name="p_sm", bufs=2))
        p_sq = ctx.enter_context(tc.tile_pool(name="p_sq", bufs=2))
        p_h = ctx.enter_context(tc.tile_pool(name="p_h", bufs=1))
        p_row = ctx.enter_context(tc.tile_pool(name="p_row", bufs=2))
        p_out = ctx.enter_context(tc.tile_pool(name="p_out", bufs=2))
        p_w = ctx.enter_context(tc.tile_pool(name="p_w", bufs=2))
        p_w1 = ctx.enter_context(tc.tile_pool(name="p_w1", bufs=1))
        p_w2 = ctx.enter_context(tc.tile_pool(name="p_w2", bufs=1))
        ps_mm = ctx.enter_context(tc.tile_pool(name="ps_mm", bufs=4, space="PSUM"))
        ps_st = ctx.enter_context(tc.tile_pool(name="ps_st", bufs=2, space="PSUM"))
        ps_bc = ctx.enter_context(tc.tile_pool(name="ps_bc", bufs=2, space="PSUM"))

        # ---- constants ----
        ones_col = consts.tile([128, 1], f16)       # value 1/D: LN sum lhsT
        nc.vector.memset(ones_col, inv_d)
        ones_row = consts.tile([1, 128], f16)       # K=1 broadcast lhsT
        nc.vector.memset(ones_row, 1.0)
        eps_t = consts.tile([1, 1], f32)
        nc.vector.memset(eps_t, EPS)
        eb2_sb = consts.tile([128, S], f16)
        nc.sync.dma_start(eb2_sb[:], eb2.ap())
        id16_sb = consts.tile([128, 128], f16)
        nc.sync.dma_start(id16_sb[:], id16.ap())

        per_layer = {}
        for l in range(L):
            d = {}
            for name, dram, w in [
                ("bq", bq_t, KD), ("bk", bk_t, KD), ("bv", bv_t, KD),
                ("bo", bo_t, KD), ("b1", b1_t, KF), ("b2", b2_t, KD),
                ("l1s", l1s_t, KD), ("l1b", l1b_t, KD),
                ("l2s", l2s_t, KD), ("l2b", l2b_t, KD),
            ]:
                t = consts.tile([128, w], f32, tag=f"{name}_{l}")
                nc.sync.dma_start(t[:], dram.ap()[l])
                d[name] = t
            per_layer[l] = d

        def load_w(dram, l, nk, width, pool, tag):
            t = pool.tile([128, nk, width], f16, tag=tag)
            nc.sync.dma_start(t[:], dram.ap()[l].rearrange("k p f -> p k f"))
            return t

        def layer_norm(s_in, gamma, beta, out_sb):
            """Feature-major LN over D: out = (s-mu)*rstd*gamma+beta (fp16)."""
            for half in range(NQ):
                sl = slice(half * 512, half * 512 + 512)
                mu_ps = ps_st.tile([1, 512], f32, tag="st")
                msq_ps = ps_st.tile([1, 512], f32, tag="st")
                for k in range(KD):
                    nc.tensor.matmul(mu_ps[:], ones_col[:], s_in[:, k, sl],
                                     start=(k == 0), stop=(k == KD - 1))
                for k in range(KD):
                    sq = p_sq.tile([128, 512], f16, tag="sq")
                    nc.scalar.activation(sq[:], s_in[:, k, sl], Act.Square)
                    nc.tensor.matmul(msq_ps[:], ones_col[:], sq[:],
                                     start=(k == 0), stop=(k == KD - 1))
                # var = msq - mu^2 ; rstd = 1/sqrt(var+eps)
                musq = p_row.tile([1, 512], f32, tag="row32")
                nc.scalar.activation(musq[:], mu_ps[:], Act.Square)
                var = p_row.tile([1, 512], f32, tag="row32")
                nc.vector.tensor_tensor(var[:], msq_ps[:], musq[:], Alu.subtract)
                std = p_row.tile([1, 512], f32, tag="row32")
                nc.scalar.activation(std[:], var[:], Act.Sqrt, bias=eps_t[:])
                rstd = p_row.tile([1, 512], f16, tag="rstd")
                with nc.allow_low_precision(reason="fp16 rstd for broadcast mm"):
                    nc.vector.reciprocal(rstd[:], std[:])
                mu16 = p_row.tile([1, 512], f16, tag="mu16")
                nc.scalar.activation(mu16[:], mu_ps[:], Act.Copy)
                # broadcast over partitions via K=1 matmuls
                mu_b = ps_bc.tile([128, 512], f32, tag="bc")
                nc.tensor.matmul(mu_b[:], ones_row[:], mu16[:],
                                 start=True, stop=True)
                rstd_b = ps_bc.tile([128, 512], f32, tag="bc")
                nc.tensor.matmul(rstd_b[:], ones_row[:], rstd[:],
                                 start=True, stop=True)
                for m in range(KD):
                    t0 = p_sq.tile([128, 512], f16, tag="lnt")
                    nc.vector.scalar_tensor_tensor(
                        t0[:], s_in[:, m, sl], 1.0, mu_b[:],
                        Alu.mult, Alu.subtract)
                    nc.vector.scalar_tensor_tensor(
                        t0[:], t0[:], gamma[:, m:m + 1], rstd_b[:],
                        Alu.mult, Alu.mult)
                    nc.vector.tensor_scalar_add(
                        out_sb[:, m, sl], t0[:], beta[:, m:m + 1])

        def dump(tile_f16, c):
            """Debug: cast a [128,*] f16 tile to f32 and DMA into y (flat)."""
            yf = y.ap().rearrange("t d -> (t d)").rearrange(
                "(p f) -> p f", p=128)
            flat = tile_f16[:]
            if len(flat.shape) == 3:
                flat = flat.rearrange("p a b -> p (a b)")
            np_, n = flat.shape
            for q in range(n // 512):
                t32 = p_out.tile([128, 512], f32, tag="dump")
                nc.scalar.activation(t32[:np_], flat[:, q * 512:(q + 1) * 512],
                                     Act.Copy)
                nc.sync.dma_start(
                    yf[:np_, c * n + q * 512: c * n + (q + 1) * 512],
                    t32[:np_])

        # ---------------- main program ----------------
        for c in range(n_chunks):
            tok0 = c * TC
            x16 = p_x.tile([128, KD, TC], f16, tag="x16")
            nc.sync.dma_start(
                x16[:], xw.ap()[:, :, tok0:tok0 + TC].rearrange("o p t -> p o t"))

            for l in range(L):
                cl = per_layer[l]
                xin = x16

                # --- Q, K projections (feature-major) ---
                wq = load_w(Wq_s, l, KD, D, p_w, "wqkvo")
                q16 = p_act.tile([128, KD, TC], f16, tag="q16")
                wk = load_w(Wk_s, l, KD, D, p_w, "wqkvo")
                k16 = p_act.tile([128, KD, TC], f16, tag="k16")
                for w_sl, out_sb, bias in ((wq, q16, cl["bq"]),
                                           (wk, k16, cl["bk"])):
                    for m in range(KD):
                        for q in range(NQ):
                            ps = ps_mm.tile([128, 512], f32, tag="mm")
                            for k in range(KD):
                                nc.tensor.matmul(
                                    ps[:],
                                    w_sl[:, k, m * 128:(m + 1) * 128],
                                    xin[:, k, q * 512:(q + 1) * 512],
                                    start=(k == 0), stop=(k == KD - 1))
                            nc.scalar.activation(
                                out_sb[:, m, q * 512:(q + 1) * 512], ps[:],
                                Act.Identity, bias=bias[:, m:m + 1], scale=1.0)

                if stage == "qk":
                    dump(q16, c)
                    break

                # --- V token-major per batch: v[s, batch, hd] (base 0) ---
                wv = load_w(Wv_s, l, KD, D, p_w, "wqkvo")
                v16a = p_act.tile([64, BC // 2, D], f16, tag="v16")
                v16b = p_h.tile([64, BC // 2, D], f16, tag="h16")

                def vslice(b, cols):
                    t = v16a if b < BC // 2 else v16b
                    return t[:, b % (BC // 2), cols]

                for b in range(BC):
                    for nh in range(2):
                        ps = ps_mm.tile([64, 384], f32, tag="mm")
                        for k in range(KD):
                            nc.tensor.matmul(
                                ps[:],
                                xin[:, k, b * 64:(b + 1) * 64],
                                wv[:, k, nh * 384:(nh + 1) * 384],
                                start=(k == 0), stop=(k == KD - 1))
                        nc.scalar.activation(
                            vslice(b, slice(nh * 384, (nh + 1) * 384)),
                            ps[:], Act.Copy)

                if stage == "v":
                    dump(v16a, c)
                    break

                # --- attention per head ---
                wo = load_w(Wo_s, l, KD, D, p_w, "wqkvo")
                ctx16 = p_act.tile([128, KD, TC], f16, tag="ctx16")
                for h in range(H):
                    # scores: 16 batches in one [128,512] psum tile
                    sc_ps = ps_mm.tile([128, 512], f32, tag="mm")
                    for b in range(BC):
                        p_slot = b % 2
                        j = b // 2
                        nc.tensor.matmul(
                            sc_ps[64 * p_slot:64 * p_slot + 64,
                                  j * 64:(j + 1) * 64],
                            q16[:, h, b * 64:(b + 1) * 64],
                            k16[:, h, b * 64:(b + 1) * 64],
                            start=True, stop=True,
                            tile_position=(0, 64 * p_slot))
                    # softmax over keys: p = exp(s)*exp(bias) / sum
                    ex = p_sm.tile([128, 8, S], f16, tag="ex")
                    nc.scalar.activation(
                        ex[:].rearrange("p a b -> p (a b)"), sc_ps[:], Act.Exp)
                    if stage == "attn_sc":
                        dump(ex, c)
                        break
                    nc.vector.tensor_tensor(
                        ex[:], ex[:],
                        eb2_sb[:, None, :].to_broadcast((128, 8, S)),
                        Alu.mult)
                    sums = p_row.tile([128, 8], f32, tag="sums")
                    nc.vector.reduce_sum(sums[:], ex[:],
                                         axis=mybir.AxisListType.X)
                    rec = p_row.tile([128, 8], f32, tag="rec")
                    nc.vector.reciprocal(rec[:], sums[:])
                    for j in range(8):
                        nc.vector.tensor_scalar_mul(
                            ex[:, j, :], ex[:, j, :], rec[:, j:j + 1])
                    if stage == "attn_sm":
                        dump(ex, c)
                        break
                    # transpose probs on PE -> pT (keys at partitions 0:63)
                    pTs = []
                    for halfj in range(2):
                        tp_ps = ps_mm.tile([128, 512], f16, tag="mm")
                        for jj in range(4):
                            j = halfj * 4 + jj
                            nc.tensor.transpose(
                                tp_ps[:64, jj * 128:(jj + 1) * 128],
                                ex[:, j, :], id16_sb[:])
                        pT = p_sm.tile([64, 512], f16, tag="pT")
                        nc.scalar.activation(pT[:], tp_ps[:64, :], Act.Copy)
                        pTs.append(pT)
                    if stage == "attn_tp":
                        dump(pTs[0], c)
                        break
                    # context: ctxT[hd, q] = v.T @ probsT, per batch
                    for half in range(NQ):
                        cx_ps = ps_mm.tile([128, 512], f32, tag="mm")
                        for bb in range(8):
                            b = half * 8 + bb
                            p_slot = b % 2
                            j = b // 2
                            pT = pTs[j // 4]
                            nc.tensor.matmul(
                                cx_ps[:, bb * 64:(bb + 1) * 64],
                                vslice(b, slice(h * 128, (h + 1) * 128)),
                                pT[:, (j % 4) * 128 + 64 * p_slot:
                                   (j % 4) * 128 + 64 * p_slot + 64],
                                start=True, stop=True)
                        nc.scalar.activation(
                            ctx16[:, h, half * 512:(half + 1) * 512],
                            cx_ps[:], Act.Identity,
                            bias=cl["bv"][:, h:h + 1], scale=1.0)

                if stage in ("attn_sc", "attn_sm", "attn_tp"):
                    break
                if stage == "attn":
                    dump(ctx16, c)
                    break

                # --- Wo + residual -> s1, then LN1 -> z16 ---
                s1 = p_act.tile([128, KD, TC], f16, tag="q16")
                for m in range(KD):
                    for q in range(NQ):
                        ps = ps_mm.tile([128, 512], f32, tag="mm")
                        for k in range(KD):
                            nc.tensor.matmul(
                                ps[:],
                                wo[:, k, m * 128:(m + 1) * 128],
                                ctx16[:, k, q * 512:(q + 1) * 512],
                                start=(k == 0), stop=(k == KD - 1))
                        nc.vector.scalar_tensor_tensor(
                            s1[:, m, q * 512:(q + 1) * 512], ps[:],
                            cl["bo"][:, m:m + 1],
                            xin[:, m, q * 512:(q + 1) * 512],
                            Alu.add, Alu.add)
                z16 = p_act.tile([128, KD, TC], f16, tag="k16")
                layer_norm(s1, cl["l1s"], cl["l1b"], z16)
                if stage == "ln1":
                    dump(z16, c)
                    break

                # --- FFN ---
                w1 = load_w(W1_s, l, KD, FF, p_w1, "w1")
                w2 = load_w(W2_s, l, KF, D, p_w2, "w2")
                last = (l == L - 1)
                if not last:
                    xout = p_x.tile([128, KD, TC], f16, tag="x16")
                s2 = p_act.tile([128, KD, TC], f16, tag="v16")
                for q in range(NQ):
                    qsl = slice(q * 512, q * 512 + 512)
                    h16 = p_h.tile([128, KF, 512], f16, tag="h16")
                    for m in range(KF):
                        ps = ps_mm.tile([128, 512], f32, tag="mm")
                        for k in range(KD):
                            nc.tensor.matmul(
                                ps[:],
                                w1[:, k, m * 128:(m + 1) * 128],
                                z16[:, k, qsl],
                                start=(k == 0), stop=(k == KD - 1))
                        nc.scalar.activation(
                            h16[:, m, :], ps[:], Act.Relu,
                            bias=cl["b1"][:, m:m + 1], scale=1.0)
                    for m in range(KD):
                        ps = ps_mm.tile([128, 512], f32, tag="mm")
                        for k in range(KF):
                            nc.tensor.matmul(
                                ps[:],
                                w2[:, k, m * 128:(m + 1) * 128],
                                h16[:, k, :],
                                start=(k == 0), stop=(k == KF - 1))
                        nc.vector.scalar_tensor_tensor(
                            s2[:, m, qsl], ps[:], cl["b2"][:, m:m + 1],
                            z16[:, m, qsl], Alu.add, Alu.add)

                if stage == "ffn":
                    dump(s2, c)
                    break

                # --- LN2 ---
                if last:
                    x2 = p_act.tile([128, KD, TC], f16, tag="ctx16")
                    layer_norm(s2, cl["l2s"], cl["l2b"], x2)
                    # transpose to token-major fp32 and store
                    for t in range(T8):
                        ps_a = ps_mm.tile([128, 512], f16, tag="mm")
                        ps_b = ps_mm.tile([128, 512], f16, tag="mm")
                        for po in range(KD):
                            tgt = ps_a if po < 4 else ps_b
                            off = (po % 4) * 128
                            nc.tensor.transpose(
                                tgt[:, off:off + 128],
                                x2[:, po, t * 128:(t + 1) * 128],
                                id16_sb[:])
                        ob = p_out.tile([128, KD, 128], f32, tag="ob")
                        nc.scalar.activation(
                            ob[:, :4, :].rearrange("p a b -> p (a b)"),
                            ps_a[:], Act.Copy)
                        nc.scalar.activation(
                            ob[:, 4:, :].rearrange("p a b -> p (a b)"),
                            ps_b[:, :256], Act.Copy)
                        nc.sync.dma_start(
                            y.ap()[tok0 + t * 128: tok0 + (t + 1) * 128, :],
                            ob[:].rearrange("p a b -> p (a b)"))
                else:
                    layer_norm(s2, cl["l2s"], cl["l2b"], xout)
                    x16 = xout

    nc.finalize()
    return nc


def _host_prep(inputs):
    x = np.asarray(inputs["x"])
    scale = 1.0 / math.sqrt(HD)
    f16 = np.float16
    f32 = np.float32

    def slabs(w, nk):
        return np.ascontiguousarray(
            np.asarray(w).reshape(L, nk, 128, np.asarray(w).shape[-1])
        ).astype(f16)

    def cols(b, nk):  # [L, feat] -> [L, 128, nk]
        return np.ascontiguousarray(
            np.asarray(b, f32).reshape(L, nk, 128).transpose(0, 2, 1))

    prep = {
        "Wq_s": (np.asarray(inputs["Wq"]) * scale)
        .reshape(L, KD, 128, D).astype(f16),
        "Wk_s": slabs(inputs["Wk"], KD),
        "Wv_s": slabs(inputs["Wv"], KD),
        "Wo_s": slabs(inputs["Wo"], KD),
        "W1_s": slabs(inputs["W1"], KD),
        "W2_s": slabs(inputs["W2"], KF),
        "bq_t": cols(np.asarray(inputs["bq"]) * scale, KD),
        "bk_t": cols(inputs["bk"], KD),
        "bv_t": cols(inputs["bv"], KD),
        "bo_t": cols(inputs["bo"], KD),
        "b1_t": cols(inputs["b1"], KF),
        "b2_t": cols(inputs["b2"], KD),
        "l1s_t": cols(inputs["ln1_s"], KD),
        "l1b_t": cols(inputs["ln1_b"], KD),
        "l2s_t": cols(inputs["ln2_s"], KD),
        "l2b_t": cols(inputs["ln2_b"], KD),
    }
    prep = {k: np.ascontiguousarray(v) for k, v in prep.items()}

    bias = np.asarray(inputs["sp_table"])[np.asarray(inputs["sp_matrix"])]
    eb = np.exp(bias.astype(np.float64)).astype(f16)
    prep["eb2"] = np.ascontiguousarray(np.concatenate([eb, eb], axis=0))
    prep["id16"] = np.eye(128, dtype=f16)

    # x: [B, S, D] -> per-core feature-major fp16 [NCORES, KD, 128, TOK]
    x16 = x.astype(f16).reshape(NCORES, TOK, KD, 128)
    xw = np.ascontiguousarray(x16.transpose(0, 2, 3, 1))
    return prep, xw


def kernel(**inputs) -> np.ndarray:
    from concourse import bass_utils

    n_chunks = int(inputs.pop("_n_chunks", TOK // TC))
    trace = bool(inputs.pop("_trace", False))

    if n_chunks not in _CACHED_NC:
        _CACHED_NC[n_chunks] = _build_nc(n_chunks)
    nc = _CACHED_NC[n_chunks]

    prep, xw = _host_prep(inputs)
    in_maps = [dict(prep, xw=np.ascontiguousarray(xw[c]))
               for c in range(NCORES)]

    res = bass_utils.run_bass_kernel_spmd(
        nc, in_maps, core_ids=list(range(NCORES)), trace=trace)
    kernel.last_result = res

    out = np.zeros((B, S, D), dtype=np.float32)
    ntok = n_chunks * TC
    for c in range(NCORES):
        yc = res.results[c]["y"][:ntok]
        out[c * BS: c * BS + ntok // S] = yc.reshape(ntok // S, S, D)
    return out

